# revision 2
# baseline (speedup 1.0000x reference)
"""LFISTA Trainium2 kernel: 16 FISTA iterations, data-parallel over batch on 8 cores.

Per core (batch chunk 128): state kept in SBUF as [128 batch, free] tiles.
Matmuls use fp16 weights (W^T and W/L resident in SBUF), stationary operand =
transposed activations (PE transpose), moving operand = weight rows (N=512).
Elementwise chain in fp32 on DVE; soft-threshold via x = v - clip(v, -t, t).
"""
import math
import numpy as np

B = 1024
S = 2048
ITERS = 16
NCORES = 8
BC = B // NCORES  # 128
NCH = S // 128    # 16 chunks
NB = S // 512     # 4 psum banks per matmul output


def _momentum_coeffs(n):
    cks = []
    t = 1.0
    for _ in range(n):
        t_new = (1.0 + math.sqrt(1.0 + 4.0 * t * t)) / 2.0
        cks.append((t - 1.0) / t_new)
        t = t_new
    return cks


def _build(invL, thresh, cks):
    import concourse.bacc as bacc
    import concourse.mybir as mybir
    from concourse.tile import TileContext
    from concourse.masks import make_identity

    dt = mybir.dt
    ALU = mybir.AluOpType
    f32, f16 = dt.float32, dt.bfloat16

    nc = bacc.Bacc("TRN2", target_bir_lowering=False, debug=False)

    src_d = nc.dram_tensor("src", [BC, S], f32, kind="ExternalInput")
    yin_d = nc.dram_tensor("yin", [BC, S], f32, kind="ExternalInput")
    wt_d = nc.dram_tensor("wt", [S, S], f16, kind="ExternalInput")   # W^T
    w2_d = nc.dram_tensor("w2", [S, S], f16, kind="ExternalInput")   # W/L
    out_d = nc.dram_tensor("out", [BC, 2 * S], f32, kind="ExternalOutput")

    with TileContext(nc) as tc:
        with tc.tile_pool(name="wpool", bufs=1) as wp, \
             tc.tile_pool(name="state", bufs=1) as st, \
             tc.tile_pool(name="work", bufs=1) as wk, \
             tc.tile_pool(name="w2s", bufs=3) as w2p, \
             tc.tile_pool(name="pmm", bufs=1, space="PSUM") as pmm, \
             tc.tile_pool(name="ptr", bufs=2, space="PSUM") as ptr:

            # ---- resident weights
            wt_sb = wp.tile([128, NCH, S], f16, name="wt_sb")
            for c in range(NCH):
                nc.sync.dma_start(wt_sb[:, c, :], wt_d[c * 128:(c + 1) * 128, :])

            # ---- inputs / state
            src = st.tile([128, S], f32, name="src")
            yin = st.tile([128, S], f32, name="yin")
            nc.sync.dma_start(src[:], src_d[:])
            nc.sync.dma_start(yin[:], yin_d[:])

            xthA = st.tile([128, S], f32, name="xthA")
            xthB = st.tile([128, S], f32, name="xthB")
            xdlA = st.tile([128, S], f32, name="xdlA")
            xdlB = st.tile([128, S], f32, name="xdlB")
            yth16 = st.tile([128, S], f16, name="yth16")
            ydl = st.tile([128, S], f32, name="ydl")
            nc.vector.memset(xthA[:], 0.0)
            nc.vector.memset(xdlA[:], 0.0)
            nc.vector.memset(ydl[:], 0.0)

            ident = st.tile([128, 128], f16, name="ident")
            make_identity(nc, ident[:])

            thT = st.tile([128, S], f16, name="thT")   # yth^T, chunk-flat
            zT = st.tile([128, S], f16, name="zT")     # z^T, chunk-flat
            z16 = st.tile([128, S], f16, name="z16")

            x_old = [xthA, xdlA]
            x_new = [xthB, xdlB]

            for k in range(ITERS):
                ck = cks[k]
                psum_m = [pmm.tile([128, 512], f32, name=f"pm{i}", tag=f"pm{i}")
                          for i in range(NB)]

                if k > 0:
                    # --- transpose yth16 -> thT (PE transpose, 4 per psum tile)
                    for g in range(4):
                        pt = ptr.tile([128, 512], f16, name="ptt", tag="ptt")
                        for u in range(4):
                            j = 4 * g + u
                            nc.tensor.transpose(
                                pt[:, u * 128:(u + 1) * 128],
                                yth16[:, j * 128:(j + 1) * 128], ident[:])
                        nc.scalar.copy(out=thT[:, g * 512:(g + 1) * 512], in_=pt[:])

                    # --- mm1: m1 = yth @ W^T  -> psum_m (4 banks)
                    for j in range(NCH):
                        for i4 in range(NB):
                            nc.tensor.matmul(
                                psum_m[i4][:],
                                lhsT=thT[:, j * 128:(j + 1) * 128],
                                rhs=wt_sb[:, j, i4 * 512:(i4 + 1) * 512],
                                start=(j == 0), stop=(j == NCH - 1))

                    # --- res = (yin - ydl) - src*m1
                    q = wk.tile([128, S], f32, name="q", tag="q")
                    for i4 in range(NB):
                        sl = slice(i4 * 512, (i4 + 1) * 512)
                        nc.vector.tensor_tensor(out=q[:, sl], in0=src[:, sl],
                                                in1=psum_m[i4][:], op=ALU.mult)
                    res = wk.tile([128, S], f32, name="res", tag="res")
                    nc.vector.tensor_tensor(out=res[:], in0=yin[:], in1=ydl[:],
                                            op=ALU.subtract)
                    nc.vector.tensor_tensor(out=res[:], in0=res[:], in1=q[:],
                                            op=ALU.subtract)
                    res_ap = res[:]
                else:
                    # y == 0 -> m1 == 0, res = yin - ydl(=0) = yin
                    res_ap = yin[:]

                # --- z = src * res (fp16 for mm2)
                nc.vector.tensor_tensor(out=z16[:], in0=src[:], in1=res_ap,
                                        op=ALU.mult)

                # --- vdl = ydl + res/L ; soft-threshold ; momentum (delta half)
                vdl = wk.tile([128, S], f32, name="vdl", tag="vdl")
                nc.vector.scalar_tensor_tensor(out=vdl[:], in0=res_ap, scalar=invL,
                                               in1=ydl[:], op0=ALU.mult, op1=ALU.add)
                cdl = wk.tile([128, S], f32, name="cdl", tag="clip")
                nc.vector.tensor_scalar(out=cdl[:], in0=vdl[:], scalar1=-thresh,
                                        scalar2=thresh, op0=ALU.max, op1=ALU.min)
                nc.vector.tensor_tensor(out=x_new[1][:], in0=vdl[:], in1=cdl[:],
                                        op=ALU.subtract)
                ddl = wk.tile([128, S], f32, name="ddl", tag="q")
                nc.vector.tensor_tensor(out=ddl[:], in0=x_new[1][:], in1=x_old[1][:],
                                        op=ALU.subtract)
                nc.vector.scalar_tensor_tensor(out=ydl[:], in0=ddl[:], scalar=ck,
                                               in1=x_new[1][:], op0=ALU.mult,
                                               op1=ALU.add)

                # --- transpose z16 -> zT
                for g in range(4):
                    pt2 = ptr.tile([128, 512], f16, name="ptz", tag="ptt")
                    for u in range(4):
                        i = 4 * g + u
                        nc.tensor.transpose(
                            pt2[:, u * 128:(u + 1) * 128],
                            z16[:, i * 128:(i + 1) * 128], ident[:])
                    nc.scalar.copy(out=zT[:, g * 512:(g + 1) * 512], in_=pt2[:])

                # --- mm2: m2 = z @ (W/L) -> psum_m (banks reused)
                psum_m2 = [pmm.tile([128, 512], f32, name=f"pn{i}", tag=f"pm{i}")
                           for i in range(NB)]
                for i in range(NCH):
                    w2c = w2p.tile([128, S], f16, name="w2c", tag="w2c")
                    nc.sync.dma_start(w2c[:], w2_d[i * 128:(i + 1) * 128, :])
                    for j4 in range(NB):
                        nc.tensor.matmul(
                            psum_m2[j4][:],
                            lhsT=zT[:, i * 128:(i + 1) * 128],
                            rhs=w2c[:, j4 * 512:(j4 + 1) * 512],
                            start=(i == 0), stop=(i == NCH - 1))

                # --- vth = yth + m2 ; soft-threshold ; momentum (theta half)
                vth = wk.tile([128, S], f32, name="vth", tag="q")
                for j4 in range(NB):
                    sl = slice(j4 * 512, (j4 + 1) * 512)
                    if k > 0:
                        nc.vector.tensor_tensor(out=vth[:, sl], in0=yth16[:, sl],
                                                in1=psum_m2[j4][:], op=ALU.add)
                    else:
                        nc.vector.tensor_copy(out=vth[:, sl], in_=psum_m2[j4][:])
                cth = wk.tile([128, S], f32, name="cth", tag="clip")
                nc.vector.tensor_scalar(out=cth[:], in0=vth[:], scalar1=-thresh,
                                        scalar2=thresh, op0=ALU.max, op1=ALU.min)
                nc.vector.tensor_tensor(out=x_new[0][:], in0=vth[:], in1=cth[:],
                                        op=ALU.subtract)
                dth = wk.tile([128, S], f32, name="dth", tag="res")
                nc.vector.tensor_tensor(out=dth[:], in0=x_new[0][:], in1=x_old[0][:],
                                        op=ALU.subtract)
                nc.vector.scalar_tensor_tensor(out=yth16[:], in0=dth[:], scalar=ck,
                                               in1=x_new[0][:], op0=ALU.mult,
                                               op1=ALU.add)

                x_old, x_new = x_new, x_old

            # final x is in x_old after the swap
            nc.sync.dma_start(out_d[:, :S], x_old[0][:])
            nc.sync.dma_start(out_d[:, S:], x_old[1][:])

    nc.finalize()
    return nc


_CACHE = {}


def _prepare(src, Y, W, alpha):
    """Build (cached) nc and the per-core input maps."""
    src = np.asarray(src)
    Y = np.asarray(Y)
    W = np.asarray(W)
    alpha = np.asarray(alpha)

    # Lipschitz constant (host): max eig of W^T W
    G = W.astype(np.float64).T @ W.astype(np.float64)
    L = float(np.linalg.eigvalsh(G)[-1])
    invL = float(np.float32(1.0 / L))
    thresh = float(np.float32(float(alpha.reshape(-1)[0]) / L * 0.5))
    cks = _momentum_coeffs(ITERS)

    key = (invL, thresh)
    if key not in _CACHE:
        _CACHE[key] = _build(invL, thresh, cks)
    nc = _CACHE[key]

    import ml_dtypes
    wt16 = np.ascontiguousarray(W.T).astype(ml_dtypes.bfloat16)
    w216 = (W / L).astype(ml_dtypes.bfloat16)
    src2 = src.reshape(B, S).astype(np.float32)
    Y2 = Y.reshape(B, S).astype(np.float32)

    in_maps = []
    for c in range(NCORES):
        sl = slice(c * BC, (c + 1) * BC)
        in_maps.append({
            "src": np.ascontiguousarray(src2[sl]),
            "yin": np.ascontiguousarray(Y2[sl]),
            "wt": wt16,
            "w2": w216,
        })
    return nc, in_maps


def build_for_profile(inputs):
    return _prepare(**inputs)


def kernel(src, Y, W, alpha):
    from concourse.bass_utils import run_bass_kernel_spmd

    nc, in_maps = _prepare(src, Y, W, alpha)
    r = run_bass_kernel_spmd(nc, in_maps, core_ids=list(range(NCORES)))
    out = np.concatenate([r.results[c]["out"] for c in range(NCORES)], axis=0)
    return out.reshape(B, 2 * S, 1).astype(np.float32)

